# revision 18
# baseline (speedup 1.0000x reference)
"""AdaptiveSAGE GNN message-passing kernel for 8 TRN2 NeuronCores.

Sharding: by DESTINATION node range (6250 dst nodes per core) so each core
exclusively owns its output slice -> no collective needed.  The host does
data movement / planning only: edge sorting, padding, index packing, and
materialization of each core's per-edge source-feature stream (a gather =
pure data movement; h rows are laid out in the order the core's edge tiles
consume them, so the device streams them sequentially at full DMA bandwidth
instead of issuing one 256B descriptor per edge).  All FLOPs (coefficient
products, message scaling, segment-sum, mean, MLP, relu) run on device.

Device pipeline per core:
  - stream hg (pre-laid-out h[src] rows, bf16) chunk by chunk
  - DVE builds a scaled one-hot per 128-edge tile in one fused op:
        oh[e, slot] = (iota[slot] == slot_e) * coeff_e,
        coeff_e = alpha[idx_e] * edge_weight_e * (1/deg[dst_e])  (mean folded)
  - TensorE: psum[dim, slot] += hg[e, dim]^T-contract oh[e, slot] (segment sum)
  - per 128-node window: MLP psum2[j, slot] = W^T @ cast_bf16(psum);
    relu(+b); DMA out
Host reassembles out[128, 6272] per core -> z[50000, 128].
"""

import sys

if "/opt/trn_rl_repo" not in sys.path:
    sys.path.insert(0, "/opt/trn_rl_repo")

import numpy as np
import ml_dtypes

import concourse.bass as bass
import concourse.bacc as bacc
import concourse.mybir as mybir
import concourse.tile as tile
from concourse.bass_utils import run_bass_kernel_spmd

N_NODES = 50000
DIM = 128
NCORES = 8
NPC = N_NODES // NCORES          # 6250 dst nodes per core
WINW = 128                       # dst-window width (one-hot/psum free dim)
NWIN = (NPC + WINW - 1) // WINW  # 49 windows of 128 dst nodes
CHUNK_TILES = 64                 # tiles per hg stream chunk (2 MB)
GP_EVERY = 1 << 30               # gpsimd one-hot offload disabled (2.3us/op, stalls chains)
P = 128

f32 = mybir.dt.float32
bf16 = mybir.dt.bfloat16


def _preprocess(h, alpha, edge_weight, W, b, node_id, edge_src, edge_dst):
    """Host-side planning: sort/pad edges, pack device images. Data movement only."""
    src = np.asarray(edge_src).astype(np.int64)
    dst = np.asarray(edge_dst).astype(np.int64)
    node_id = np.asarray(node_id).astype(np.int64)
    alpha = np.asarray(alpha, dtype=np.float32)
    ew = np.asarray(edge_weight, dtype=np.float32)
    E = src.shape[0]
    gene_num = alpha.shape[0] - 2

    src_id = node_id[src]
    dst_id = node_id[dst]
    gi = np.full(E, gene_num + 1, np.int64)
    gi = np.where((src_id >= 0) & (dst_id < 0), src_id, gi)
    gi = np.where((dst_id >= 0) & (src_id < 0), dst_id, gi)
    gi = np.where((dst_id >= 0) & (src_id >= 0), gene_num, gi)
    a_e = alpha[gi]                                   # gather (data movement)

    deg = np.bincount(dst, minlength=N_NODES).astype(np.float32)
    cnt_e = np.maximum(deg[dst], 1.0)                 # metadata gather

    core = dst // NPC
    ldst = dst - core * NPC
    w_id = ldst // WINW
    slot = (ldst % WINW).astype(np.float32)

    # group key: (core, window)
    key = core * NWIN + w_id
    order = np.argsort(key, kind="stable")
    ncount = np.bincount(key, minlength=NCORES * NWIN).reshape(NCORES, NWIN)

    # common (max-over-cores) tile counts per window -> static SPMD schedule
    T = np.maximum(np.ceil(ncount / P).astype(np.int64).max(axis=0), 1)  # [NWIN]
    TT = int(T.sum())
    EP = TT * P

    tile_off = np.zeros(NWIN, np.int64)
    tile_off[1:] = np.cumsum(T)[:-1]

    key_sorted = key[order]
    grp_start = np.zeros(NCORES * NWIN, np.int64)
    grp_start[1:] = np.cumsum(ncount.reshape(-1))[:-1]
    rank = np.arange(E, dtype=np.int64) - grp_start[key_sorted]
    w_sorted = key_sorted % NWIN
    core_sorted = key_sorted // NWIN
    pos = P * tile_off[w_sorted] + rank

    gidx_p = np.zeros((NCORES, EP), np.int32)
    slot_p = np.zeros((NCORES, EP), np.float32)
    a_p = np.zeros((NCORES, EP), np.float32)
    w_p = np.zeros((NCORES, EP), np.float32)
    cnt_p = np.ones((NCORES, EP), np.float32)
    gidx_p[core_sorted, pos] = src[order].astype(np.int32)
    slot_p[core_sorted, pos] = slot[order]
    a_p[core_sorted, pos] = a_e[order]
    w_p[core_sorted, pos] = ew[order]
    cnt_p[core_sorted, pos] = cnt_e[order]

    # images: edge pos = t*128 + p  ->  [p, t]
    def img(x):
        return np.ascontiguousarray(x.reshape(NCORES, TT, P).transpose(0, 2, 1))

    h_bf = np.asarray(h, np.float32).astype(ml_dtypes.bfloat16)
    # per-core source-feature stream, laid out exactly as consumed:
    # [128 partitions, TT tiles, DIM] with edge (t, p) at [p, t, :]
    hg_img = np.ascontiguousarray(
        h_bf[gidx_p.reshape(NCORES, TT, P)].transpose(0, 2, 1, 3))

    plan = dict(
        T=T, TT=TT, EP=EP, tile_off=tile_off,
        hg_img=hg_img, slot_img=img(slot_p), a_img=img(a_p),
        w_img=img(w_p), cnt_img=img(cnt_p),
        idx_img=img(gidx_p),
        wt_bf=np.ascontiguousarray(np.asarray(W, np.float32).T).astype(ml_dtypes.bfloat16),
        b_col=np.ascontiguousarray(np.asarray(b, np.float32).reshape(DIM, 1)),
    )
    return plan


def _build(plan):
    """Build the (SPMD-identical) Bass graph from the static plan."""
    T = plan["T"]
    TT = plan["TT"]
    tile_off = plan["tile_off"]

    nc = bacc.Bacc("TRN2", target_bir_lowering=False, debug=False,
                   num_swdge_queues=4)
    hg_d = nc.dram_tensor("hgimg", [P, TT, DIM], bf16, kind="ExternalInput")
    slot_d = nc.dram_tensor("slotimg", [P, TT], f32, kind="ExternalInput")
    a_d = nc.dram_tensor("aimg", [P, TT], f32, kind="ExternalInput")
    w_d = nc.dram_tensor("wimg", [P, TT], f32, kind="ExternalInput")
    cnt_d = nc.dram_tensor("cntimg", [P, TT], f32, kind="ExternalInput")
    wt_d = nc.dram_tensor("wt", [DIM, DIM], bf16, kind="ExternalInput")
    b_d = nc.dram_tensor("bvec", [DIM, 1], f32, kind="ExternalInput")
    out_d = nc.dram_tensor("out", [P, NWIN * WINW], f32, kind="ExternalOutput")

    with tile.TileContext(nc) as tc:
        with (
            tc.tile_pool(name="const", bufs=1) as cpool,
            tc.tile_pool(name="gather", bufs=3) as gpool,
            tc.tile_pool(name="oh", bufs=6) as ohpool,
            tc.tile_pool(name="mlp", bufs=3) as mpool,
            tc.tile_pool(name="psum", bufs=4, space="PSUM") as pspool,
            tc.tile_pool(name="psum2", bufs=2, space="PSUM") as ps2pool,
        ):
            iota_f = cpool.tile([P, P], f32, tag="iotaf")
            nc.gpsimd.iota(iota_f[:], pattern=[[1, P]], base=0,
                           channel_multiplier=0,
                           allow_small_or_imprecise_dtypes=True)
            iota_sb = cpool.tile([P, P], bf16, tag="iota")
            nc.vector.tensor_copy(out=iota_sb[:], in_=iota_f[:])

            # coeff = a * w * (1/cnt), pipelined in two SEPARATE tile objects
            # (Tile deps are per tile, so the head segment unblocks the first
            # windows without waiting for the full-width chain)
            SEG = min(128, TT)
            segs = [(0, SEG)] + ([(SEG, TT)] if TT > SEG else [])
            slot_t, coeff_t = [], []

            def emit_coeff_seg(lo, hi):
                n = hi - lo
                sl = slice(lo, hi)
                cnt_s = cpool.tile([P, n], f32, tag=f"cnt{lo}")
                nc.sync.dma_start(cnt_s[:], cnt_d.ap()[:, sl])
                a_s = cpool.tile([P, n], f32, tag=f"a{lo}")
                nc.sync.dma_start(a_s[:], a_d.ap()[:, sl])
                w_s = cpool.tile([P, n], f32, tag=f"w{lo}")
                nc.sync.dma_start(w_s[:], w_d.ap()[:, sl])
                sl_s = cpool.tile([P, n], f32, tag=f"slot{lo}")
                nc.sync.dma_start(sl_s[:], slot_d.ap()[:, sl])
                r_s = cpool.tile([P, n], f32, tag=f"r{lo}")
                nc.vector.reciprocal(r_s[:], cnt_s[:])
                c_s = cpool.tile([P, n], f32, tag=f"c{lo}")
                nc.vector.tensor_tensor(out=c_s[:], in0=a_s[:], in1=w_s[:],
                                        op=mybir.AluOpType.mult)
                nc.vector.tensor_tensor(out=c_s[:], in0=c_s[:], in1=r_s[:],
                                        op=mybir.AluOpType.mult)
                slot_t.append(sl_s)
                coeff_t.append(c_s)

            emit_coeff_seg(*segs[0])

            def seg_col(t):
                return (0, t) if t < SEG else (1, t - SEG)

            wt_sb = cpool.tile([DIM, DIM], bf16, tag="wt")
            nc.sync.dma_start(wt_sb[:], wt_d.ap()[:])
            b_sb = cpool.tile([DIM, 1], f32, tag="b")
            nc.sync.dma_start(b_sb[:], b_d.ap()[:])

            stream_tiles = {}

            def ensure_streamed(ci):
                if ci in stream_tiles:
                    return stream_tiles[ci]
                t0 = ci * CHUNK_TILES
                nt = min(CHUNK_TILES, TT - t0)
                hg = gpool.tile([P, CHUNK_TILES, DIM], bf16, tag="hg")
                nc.sync.dma_start(hg[:, :nt, :], hg_d.ap()[:, t0:t0 + nt, :])
                stream_tiles[ci] = hg
                return hg

            for w in range(NWIN):
                nt_w = int(T[w])
                t0 = int(tile_off[w])
                # emit the tail coefficient segment once the upcoming windows
                # will need columns >= SEG (after the early windows started)
                if len(segs) > 1 and len(coeff_t) == 1 and t0 + nt_w + 2 * P // 3 > SEG:
                    emit_coeff_seg(*segs[1])
                psum = pspool.tile([P, WINW], f32, tag="ps")
                for k in range(nt_w):
                    t = t0 + k
                    hg = ensure_streamed(t // CHUNK_TILES)
                    kk = t % CHUNK_TILES
                    oh = ohpool.tile([P, WINW], bf16, tag="oh")
                    si, tc_ = seg_col(t)
                    nc.vector.tensor_scalar(
                        out=oh[:], in0=iota_sb[:, :WINW],
                        scalar1=slot_t[si][:, tc_: tc_ + 1],
                        scalar2=coeff_t[si][:, tc_: tc_ + 1],
                        op0=mybir.AluOpType.is_equal,
                        op1=mybir.AluOpType.mult,
                    )
                    nc.tensor.matmul(
                        psum[:], hg[:, kk, :], oh[:],
                        start=(k == 0), stop=(k == nt_w - 1),
                    )
                wsl = slice(w * WINW, (w + 1) * WINW)
                nbf = mpool.tile([P, WINW], bf16, tag="nbf")
                nc.vector.tensor_copy(out=nbf[:], in_=psum[:])
                psum2 = ps2pool.tile([P, WINW], f32, tag="ps2")
                nc.tensor.matmul(psum2[:], wt_sb[:], nbf[:], start=True, stop=True)
                zt = mpool.tile([P, WINW], f32, tag="zt")
                nc.scalar.activation(zt[:], psum2[:],
                                     mybir.ActivationFunctionType.Relu,
                                     bias=b_sb[:, :1])
                nc.sync.dma_start(out_d.ap()[:, wsl], zt[:])

    nc.compile()
    return nc


def _in_maps(plan):
    maps = []
    for c in range(NCORES):
        maps.append({
            "hgimg": plan["hg_img"][c],
            "slotimg": plan["slot_img"][c],
            "aimg": plan["a_img"][c],
            "wimg": plan["w_img"][c],
            "cntimg": plan["cnt_img"][c],
            "wt": plan["wt_bf"],
            "bvec": plan["b_col"],
        })
    return maps


_NC_CACHE = {}


def _get_nc(plan):
    key = (plan["TT"], tuple(plan["T"]))
    if key not in _NC_CACHE:
        _NC_CACHE[key] = _build(plan)
    return _NC_CACHE[key]


def kernel(**inputs):
    plan = _preprocess(**{k: np.asarray(v) for k, v in inputs.items()})
    nc = _get_nc(plan)
    res = run_bass_kernel_spmd(nc, _in_maps(plan), core_ids=list(range(NCORES)))
    z = np.empty((N_NODES, DIM), np.float32)
    for c in range(NCORES):
        z[c * NPC:(c + 1) * NPC] = res.results[c]["out"][:, :NPC].T
    return z
